# revision 20
# baseline (speedup 1.0000x reference)
"""Trainium2 Bass kernel for nn_LDS_LR: low-rank LDS + AR low-rank correction.

Math (per batch b):
    Bu   = X @ B1 @ B2                      # [T, N] rank-64 input projection
    h_t  = A * h_{t-1} + Bu_t               # diagonal recurrence, h_{-1} = h0
    lds  = H @ C1 @ C2                      # [T, O] rank-64 output projection
    proj = einsum('ti,rik->trk', X, M1)     # [T, R, KX]
    ar_t = sum_k M2[:,:,k] @ proj[t-k,:,k]  # AR with KX=5 taps
    Y    = lds + ar

Sharding: 8 cores = 4 batches x 2 sequence halves (1024 steps each),
uniform SPMD, no cross-core communication.

Carry (chunk-1 cores): h_1023 = sum_s A^(1023-s) Bu_prev[s] + A^1024 h0 is
computed WITHOUT any DVE elementwise pass over [N, T] via the rank-space
"Q-trick":  Q = g_prev @ APrev  (PE matmul, g_prev = X_prev @ B1,
APrev[s, n] = A[n]^(1023-s) from 8 Exp activations with per-partition
iota scales), then carry[n] = sum_r B2[r, n] * Q[r, n] (one small DVE
multiply [64, T] + a ones-stationary matmul partition-reduce), reshaped
to per-n-tile columns through a tiny DRAM round-trip.

DVE runs ONLY the 8 own-chunk scans (the true serial recurrence),
split into quarter-batches so CH1/Y of earlier quarters overlap later
scans; the A multiplier is a stride-0 broadcast AP (no materialized
broadcast tensor).

Dtypes: X / W1 / W2 / B2 / pext / gsb / APrev are bf16; scans keep f32
state and H / C1 matmuls run in f32r (exact f32 bits, 1 cycle/row at
free >= 256).
"""

import contextlib
import ctypes
import os
import sys
import types

import numpy as np
from contextlib import ExitStack

import concourse.bass as bass
import concourse.tile as tile
from concourse import bacc, mybir
from concourse.bass_utils import run_bass_kernel_spmd


def _install_ntff_hook():
    try:
        from antenv.axon_hooks import get_axon_ntff_profile_hook  # noqa: F401
        return
    except ImportError:
        pass
    so_path = "/opt/axon/libaxon_pjrt.so"
    hook = None
    if os.path.exists(so_path):
        lib = ctypes.CDLL(so_path)
        if hasattr(lib, "axon_start_nrt_profile"):
            lib.axon_start_nrt_profile.argtypes = [
                ctypes.POINTER(ctypes.c_int64), ctypes.c_size_t]
            lib.axon_start_nrt_profile.restype = ctypes.c_int64
            lib.axon_stop_nrt_profile.argtypes = [ctypes.c_char_p]
            lib.axon_stop_nrt_profile.restype = ctypes.c_int64

            @contextlib.contextmanager
            def _hook(output_dir, device_ids):
                import jax
                jax.devices()
                if device_ids:
                    ids = (ctypes.c_int64 * len(device_ids))(*device_ids)
                    rc = lib.axon_start_nrt_profile(ids, len(device_ids))
                else:
                    rc = lib.axon_start_nrt_profile(None, 0)
                if rc != 0:
                    raise RuntimeError(f"axon_start_nrt_profile rc={rc}")
                try:
                    yield
                finally:
                    n = lib.axon_stop_nrt_profile(str(output_dir).encode())
                    print(f"ntff profile: {n} file(s) -> {output_dir}",
                          file=sys.stderr)

            hook = _hook
    mod = types.ModuleType("antenv.axon_hooks")
    mod.get_axon_ntff_profile_hook = lambda: hook
    mod.set_axon_ntff_profile_hook = lambda h: None
    sys.modules["antenv.axon_hooks"] = mod


_install_ntff_hook()

DT = mybir.dt.float32
HDT = mybir.dt.float32r
MDT = mybir.dt.bfloat16
MNP = mybir.dt.np(MDT)
F32 = np.float32

B, T, D = 4, 2048, 1024
NST, R, KX, OUT = 1024, 64, 5, 1024
TC = 1024          # per-core chunk length
TBL = 512          # pext/ysb half block
TQ = 256           # scan / CH1 / Y quarter block
NT = 2
NQ = 4
PW = 4 + TC + 4

_CACHED_NC = None
LAST_RESULT = None

MULT = mybir.AluOpType.mult
ADD = mybir.AluOpType.add
EXP = mybir.ActivationFunctionType.Exp
LN = mybir.ActivationFunctionType.Ln


def _emit(ctx, tc, io):
    nc = tc.nc
    xo, xp, w1, w2, b2, c1, lnb, yt = io

    wp = ctx.enter_context(tc.tile_pool(name="wp", bufs=1))
    xpool = ctx.enter_context(tc.tile_pool(name="xpool", bufs=1))
    hp = ctx.enter_context(tc.tile_pool(name="hp", bufs=1))
    pp = ctx.enter_context(tc.tile_pool(name="pp", bufs=1))
    yp = ctx.enter_context(tc.tile_pool(name="yp", bufs=8))
    # PSUM (8 banks): psA(2): jp pair -> Q pair -> CH1 quarters
    #                 psB(2): jo/j1/j2 -> Y quarters
    #                 psBu(3): Bu_own quarters   psT(1): transposes/tails/colsum
    psA = ctx.enter_context(tc.tile_pool(name="psA", bufs=2, space="PSUM"))
    psB = ctx.enter_context(tc.tile_pool(name="psB", bufs=2, space="PSUM"))
    psBu = ctx.enter_context(tc.tile_pool(name="psBu", bufs=3, space="PSUM"))
    psT = ctx.enter_context(tc.tile_pool(name="psT", bufs=1, space="PSUM"))

    # ---- DMA (3 queues; X packed as [128, 8k*1024] with 2 transfers) ------
    # sync: s, ident, xpA, xpB, cscr round-trip, yt stores
    # scalar HWDGE: lnbc, w1, xoA, xoB (transfers overlap the Exp chain)
    # gpsimd SWDGE: b2, c1, w2
    c1sb = wp.tile([128, 8 * 64 + 40], HDT, tag="c1", name="c1sb")
    nc.sync.dma_start(c1sb[:], c1[:])
    xpall = xpool.tile([128, 8 * TC], MDT, tag="xpall", name="xpall")
    nc.sync.dma_start(xpall[:, 0:4 * TC], xp[:, 0:4 * TC])
    nc.sync.dma_start(xpall[:, 4 * TC:8 * TC], xp[:, 4 * TC:8 * TC])
    lnbc = wp.tile([128, TC], MDT, tag="lnbc", name="lnbc")
    nc.scalar.dma_start(lnbc[:], lnb[:])
    w1sb = wp.tile([128, 8 * 384 + 64], MDT, tag="w1", name="w1sb")
    nc.scalar.dma_start(w1sb[:], w1[:])
    xoall = xpool.tile([128, 8 * TC], MDT, tag="xoall", name="xoall")
    nc.scalar.dma_start(xoall[:, 0:4 * TC], xo[:, 0:4 * TC])
    nc.sync.dma_start(xoall[:, 4 * TC:8 * TC], xo[:, 4 * TC:8 * TC])
    b2sb = wp.tile([64, NST], MDT, tag="b2", name="b2sb")
    nc.gpsimd.dma_start(b2sb[:], b2[:])
    w2sb = wp.tile([128, 3 * 1024], MDT, tag="w2", name="w2sb")
    nc.gpsimd.dma_start(w2sb[:], w2[:])
    xpsb = [xpall[:, k * TC:(k + 1) * TC] for k in range(8)]

    def xos(k, t):  # xo is packed half-major: [t-half][k-tile][512]
        return xoall[:, t * 8 * TBL + k * TBL:t * 8 * TBL + (k + 1) * TBL]

    def w1s(k, j):
        return w1sb[:, k * 384 + j * 128: k * 384 + (j + 1) * 128]

    smf = wp.tile([128, 32], DT, tag="smf", name="smf")
    nc.vector.tensor_copy(smf[:], c1sb[:, 512:544])
    av = smf[:, 0:8]
    ivS = smf[:, 8:16]
    ioS = smf[:, 16:24]
    iotf = smf[:, 24:32]   # 1023 - 128*s - p
    idsb = w1sb[0:64, 3072:3136]

    # ---- tiny prep --------------------------------------------------------
    lnA = wp.tile([128, 8], DT, tag="lnA", name="lnA")
    nc.scalar.activation(lnA[:], av, LN)
    ap1024 = wp.tile([128, 8], DT, tag="ap1024", name="ap1024")
    nc.scalar.activation(ap1024[:], lnA[:], EXP, scale=1024.0)
    ivterm = wp.tile([128, 8], DT, tag="ivterm", name="ivterm")
    nc.vector.scalar_tensor_tensor(ivterm[:], ap1024[:], 1.0, ivS, MULT, MULT)
    nc.vector.scalar_tensor_tensor(ivterm[:], ivterm[:], 1.0, ioS, MULT, ADD)

    # ---- scalar: APrev_s[p, n] = A[n]^(1023 - 128 s - p)  (bf16) ----------
    # pext0 copies are wedged in after exp4 (jo is ready by then) so the
    # Bu_own matmuls aren't gated on the end of the exp chain.
    aprev = [wp.tile([128, TC], MDT, tag=f"apv{s}", name=f"apv{s}")
             for s in range(8)]
    pext = [pp.tile([128, PW], MDT, tag=f"pext{j}", name=f"pext{j}")
            for j in range(3)]

    # ---- PE phase 1: j0 (all jp first so gsb/carry start early) ----------
    jp = [psA.tile([128, TBL], DT, tag="pa", name=f"jp{t}") for t in range(NT)]
    jo = [psB.tile([128, TBL], DT, tag="pb", name=f"jo{t}") for t in range(NT)]
    for k in range(8):
        fl = dict(start=(k == 0), stop=(k == 7), skip_group_check=True)
        for t in range(NT):
            nc.tensor.matmul(jp[t][:], w1s(k, 0),
                             xpsb[k][:, t * TBL:(t + 1) * TBL], **fl)
    for k in range(8):
        nc.tensor.matmul(jo[0][:], w1s(k, 0), xos(k, 0),
                         start=(k == 0), stop=(k == 7))

    # scalar: APrev_s[p, n] = A[n]^(1023 - 128 s - p)  (bf16)
    for s in range(8):
        nc.scalar.activation(aprev[s][:], lnbc[:], EXP,
                             scale=iotf[:, s:s + 1])

    # DVE copies: gsb (j0-prev rows 0:64), pext0 (j0-own), gsbT
    gsb = wp.tile([64, TC], MDT, tag="gsb", name="gsb")
    for t in range(NT):
        nc.vector.tensor_copy(gsb[:, t * TBL:(t + 1) * TBL], jp[t][0:64, :])
    nc.vector.tensor_copy(pext[0][:, 4:4 + TBL], jo[0][:])

    gsbT = []
    for s in range(8):
        pt = psT.tile([128, 64], MDT, tag="tl", name=f"tp{s}")
        nc.tensor.matmul(pt[:], gsb[:, s * 128:(s + 1) * 128], idsb[:],
                         is_transpose=True)
        st = wp.tile([128, 64], MDT, tag=f"gT{s}", name=f"gT{s}")
        nc.vector.tensor_copy(st[:], pt[:])
        gsbT.append(st)

    # Q[r, n] = sum_s gsbT_s^T @ APrev_s  -> psA pair [64, 512]
    qps = [psA.tile([64, TBL], DT, tag="pa", name=f"q{h}") for h in range(2)]
    for h in range(2):
        for s in range(8):
            nc.tensor.matmul(qps[h][:], gsbT[s][:],
                             aprev[s][:, h * TBL:(h + 1) * TBL],
                             start=(s == 0), stop=(s == 7))
    qb = wp.tile([64, TC], MDT, tag="qb", name="qb")
    for h in range(2):
        nc.vector.scalar_tensor_tensor(
            qb[:, h * TBL:(h + 1) * TBL], qps[h][:], 1.0,
            b2sb[:, h * TBL:(h + 1) * TBL], MULT, MULT)
    ones64 = wp.tile([64, 1], MDT, tag="o64", name="ones64")
    nc.vector.memset(ones64[:], 1.0)

    # colsum -> csf -> 8 tiny PE transposes -> iownF [128, 8]
    csf = wp.tile([1, TC], MDT, tag="csf", name="csf")
    for h in range(2):
        cp = psT.tile([1, TBL], DT, tag="tl", name=f"cs{h}")
        nc.tensor.matmul(cp[:], ones64[:], qb[:, h * TBL:(h + 1) * TBL],
                         start=True, stop=True)
        nc.scalar.copy(csf[:, h * TBL:(h + 1) * TBL], cp[:])
    iownT = wp.tile([128, 8], DT, tag="iownT", name="iownT")
    for n in range(8):
        rp = psT.tile([128, 1], MDT, tag="tl", name=f"rt{n}")
        nc.tensor.matmul(rp[:], csf[0:1, n * 128:(n + 1) * 128],
                         idsb[0:1, 0:1], is_transpose=True)
        nc.vector.tensor_copy(iownT[:, n:n + 1], rp[:])
    iownF = wp.tile([128, 8], DT, tag="iownF", name="iownF")
    nc.vector.scalar_tensor_tensor(iownF[:], iownT[:], 1.0, ivterm[:],
                                   MULT, ADD)

    # ---- Bu_own + scans start as soon as jo/pext0 land --------------------
    hsb = [hp.tile([128, TC], HDT, tag=f"h{n}", name=f"h{n}") for n in range(8)]
    buo = {}

    def emit_buo(n, q):
        p = psBu.tile([128, TQ], DT, tag="bu", name=f"buo{n}_{q}")
        nc.tensor.matmul(p[:], b2sb[:, n * 128:(n + 1) * 128],
                         pext[0][0:64, 4 + q * TQ:4 + (q + 1) * TQ],
                         start=True, stop=True)
        buo[(n, q)] = p

    def emit_scan(n, q):
        init = iownF[:, n:n + 1] if q == 0 else hsb[n][:, q * TQ - 1:q * TQ]
        nc.vector.tensor_tensor_scan(
            hsb[n][:, q * TQ:(q + 1) * TQ],
            av[:, n:n + 1].broadcast_to([128, TQ]),
            buo[(n, q)][:], init, MULT, ADD)

    for n in range(8):
        emit_buo(n, 0)

    # ---- j1/j2/tails (needed only by Y, from ~q0 CH1 time) ----------------
    j1 = [psB.tile([128, TBL], DT, tag="pb", name=f"j1_{t}") for t in range(NT)]
    j2 = [psB.tile([128, TBL], DT, tag="pb", name=f"j2_{t}") for t in range(NT)]
    tails = psT.tile([128, 8], DT, tag="tl", name="tails")

    def emit_j(jt, j, t):
        for k in range(8):
            nc.tensor.matmul(jt[t][:], w1s(k, j), xos(k, t),
                             start=(k == 0), stop=(k == 7))

    def emit_tail(j):
        for k in range(8):
            nc.tensor.matmul(tails[:, (j - 1) * 4:j * 4], w1s(k, j),
                             xpsb[k][:, TC - 4:TC],
                             start=(k == 0), stop=(k == 7))

    emit_j(j1, 1, 0)
    emit_tail(1)
    emit_j(j2, 2, 0)
    emit_tail(2)

    for n in range(8):
        emit_scan(n, 0)
        emit_buo(n, 1)

    # scalar: pext copies for j1/j2 (t0 half) + AR boundary tails
    nc.scalar.copy(pext[1][0:64, 5:5 + TBL], j1[0][0:64, :])
    nc.scalar.copy(pext[1][64:128, 6:6 + TBL], j1[0][64:128, :])
    nc.scalar.copy(pext[2][0:64, 7:7 + TBL], j2[0][0:64, :])
    nc.scalar.copy(pext[2][64:128, 8:8 + TBL], j2[0][64:128, :])
    nc.scalar.copy(pext[1][0:64, 4:5], tails[0:64, 3:4])
    nc.scalar.copy(pext[1][64:128, 4:6], tails[64:128, 2:4])
    nc.scalar.copy(pext[2][0:64, 4:7], tails[0:64, 5:8])
    nc.scalar.copy(pext[2][64:128, 4:8], tails[64:128, 4:8])

    # ---- per-quarter: remaining scans, CH1, Y -----------------------------
    ysb = {}
    for q in range(NQ):
        if q == 1:
            # second-half inputs: PE fills the gap after Yq0; scalar copies
            # land well before Yq2 needs them
            for k in range(8):
                nc.tensor.matmul(jo[1][:], w1s(k, 0), xos(k, 1),
                                 start=(k == 0), stop=(k == 7))
            emit_j(j1, 1, 1)
            emit_j(j2, 2, 1)
            nc.scalar.copy(pext[1][0:64, 5 + TBL:5 + TC], j1[1][0:64, :])
            nc.scalar.copy(pext[1][64:128, 6 + TBL:6 + TC], j1[1][64:128, :])
            nc.scalar.copy(pext[2][0:64, 7 + TBL:7 + TC], j2[1][0:64, :])
            nc.scalar.copy(pext[2][64:128, 8 + TBL:8 + TC], j2[1][64:128, :])
        if q == 2:
            nc.vector.tensor_copy(pext[0][:, 4 + TBL:4 + TC], jo[1][:])
        if q == 1:
            for n in range(8):
                emit_scan(n, q)
        elif q >= 2:
            for n in range(8):
                emit_buo(n, q)
                emit_scan(n, q)
        c_ps = psA.tile([64, TQ], DT, tag="pa", name=f"c_ps{q}")
        for n in range(8):
            nc.tensor.matmul(c_ps[:], c1sb[:, n * 64:(n + 1) * 64],
                             hsb[n][:, q * TQ:(q + 1) * TQ],
                             start=(n == 0), stop=(n == 7))
        nc.scalar.copy(pext[0][0:64, 4 + q * TQ:4 + (q + 1) * TQ], c_ps[:])

        for o in range(8):
            y_ps = psB.tile([128, TQ], DT, tag="pb", name=f"y_ps{o}_{q}")
            for m in range(3):
                nc.tensor.matmul(y_ps[:],
                                 w2sb[:, m * 1024 + o * 128:
                                      m * 1024 + (o + 1) * 128],
                                 pext[m][:, 4 + q * TQ:4 + (q + 1) * TQ],
                                 start=(m == 0), stop=(m == 2))
            half, qh = divmod(q, 2)
            if qh == 0:
                ysb[(o, half)] = yp.tile([128, TBL], MDT, tag="y",
                                         name=f"y{o}_{half}")
            dst = ysb[(o, half)][:, qh * TQ:(qh + 1) * TQ]
            if q == 3 and o % 2 == 1:
                nc.vector.tensor_copy(dst, y_ps[:])
            else:
                nc.scalar.copy(dst, y_ps[:])
            if qh == 1:
                nc.sync.dma_start(
                    yt[o * 128:(o + 1) * 128, half * TBL:(half + 1) * TBL],
                    ysb[(o, half)][:])


def _build():
    nc = bacc.Bacc("TRN2", target_bir_lowering=False, debug=False,
                   num_devices=8)
    xo = nc.dram_tensor("xo", [128, 8 * TC], MDT, kind="ExternalInput").ap()
    xp = nc.dram_tensor("xp", [128, 8 * TC], MDT, kind="ExternalInput").ap()
    w1 = nc.dram_tensor("w1", [128, 8 * 384 + 64], MDT, kind="ExternalInput").ap()
    w2 = nc.dram_tensor("w2", [128, 3 * 1024], MDT, kind="ExternalInput").ap()
    b2 = nc.dram_tensor("b2", [R, NST], MDT, kind="ExternalInput").ap()
    c1 = nc.dram_tensor("c1", [128, 8 * 64 + 40], HDT, kind="ExternalInput").ap()
    lnb = nc.dram_tensor("lnb", [128, TC], MDT, kind="ExternalInput").ap()
    yt = nc.dram_tensor("yt", [OUT, TC], MDT, kind="ExternalOutput").ap()

    with tile.TileContext(nc) as tc, ExitStack() as ctx:
        _emit(ctx, tc, (xo, xp, w1, w2, b2, c1, lnb, yt))
    nc.compile()
    return nc


def _get_nc():
    global _CACHED_NC
    if _CACHED_NC is None:
        _CACHED_NC = _build()
    return _CACHED_NC


def _tile_pack(a, p=128):
    k = a.shape[0] // p
    return np.ascontiguousarray(
        np.transpose(a.reshape(k, p, -1), (1, 0, 2)).reshape(p, -1))


def kernel(inputs, h0, A, B1, B2, C1, C2, M1, M2):
    global LAST_RESULT
    X = np.asarray(inputs, dtype=F32)
    h0 = np.asarray(h0, dtype=F32)
    A = np.asarray(A, dtype=F32)
    W1 = np.concatenate(
        [np.asarray(B1, dtype=F32)]
        + [np.ascontiguousarray(np.asarray(M1, dtype=F32)[:, :, k].T)
           for k in range(KX)], axis=1)
    W2 = np.concatenate(
        [np.asarray(C2, dtype=F32)]
        + [np.ascontiguousarray(np.asarray(M2, dtype=F32)[:, :, k].T)
           for k in range(KX)], axis=0)
    w1c = np.concatenate(
        [_tile_pack(W1).astype(MNP),
         np.vstack([np.eye(64, dtype=F32),
                    np.zeros((64, 64), F32)]).astype(MNP)], axis=1)
    w2c = _tile_pack(W2).astype(MNP)
    b2c = np.ascontiguousarray(np.asarray(B2, dtype=F32)).astype(MNP)
    c1c = _tile_pack(np.asarray(C1, dtype=F32))
    avT = _tile_pack(A.reshape(-1, 1))
    h0T = _tile_pack(h0.reshape(-1, 1))
    zT = np.zeros((128, 8), F32)
    s8 = np.arange(8)
    p128 = np.arange(128)
    iotf = (1023.0 - 128.0 * s8[None, :] - p128[:, None]).astype(F32)
    lnbc = np.ascontiguousarray(
        np.broadcast_to(np.log(A)[None, :], (128, NST))).astype(MNP)

    xz = np.zeros((128, 8 * TC), MNP)
    in_maps = []
    for c in range(8):
        b, half = divmod(c, 2)
        XT = np.ascontiguousarray(X[b, half * TC:(half + 1) * TC, :].T)
        xoc = np.concatenate(
            [_tile_pack(np.ascontiguousarray(XT[:, h * TBL:(h + 1) * TBL]))
             for h in range(2)], axis=1).astype(MNP)
        if half == 0:
            xpc = xz
            ivT, ioT = zT, h0T
        else:
            xpc = _tile_pack(
                np.ascontiguousarray(X[b, 0:TC, :].T)).astype(MNP)
            ivT, ioT = h0T, zT
        ones8 = np.ones((128, 8), F32)
        c1x = np.ascontiguousarray(
            np.concatenate([c1c, avT, ivT, ioT, iotf, ones8], axis=1))
        in_maps.append({"xo": xoc, "xp": xpc, "w1": w1c, "w2": w2c,
                        "b2": b2c, "c1": c1x, "lnb": lnbc})

    nc = _get_nc()
    trace = bool(int(os.environ.get("KERNEL_TRACE", "0")))
    LAST_RESULT = run_bass_kernel_spmd(nc, in_maps, core_ids=list(range(8)),
                                       trace=trace)
    Y = np.empty((B, T, OUT), F32)
    for c in range(8):
        b, half = divmod(c, 2)
        Y[b, half * TC:(half + 1) * TC, :] = \
            LAST_RESULT.results[c]["yt"].T.astype(F32)
    return Y


# revision 21
# speedup vs baseline: 1.0533x; 1.0533x over previous
"""Trainium2 Bass kernel for nn_LDS_LR: low-rank LDS + AR low-rank correction.

Math (per batch b):
    Bu   = X @ B1 @ B2                      # [T, N] rank-64 input projection
    h_t  = A * h_{t-1} + Bu_t               # diagonal recurrence, h_{-1} = h0
    lds  = H @ C1 @ C2                      # [T, O] rank-64 output projection
    proj = einsum('ti,rik->trk', X, M1)     # [T, R, KX]
    ar_t = sum_k M2[:,:,k] @ proj[t-k,:,k]  # AR with KX=5 taps
    Y    = lds + ar

Sharding: 8 cores = 4 batches x 2 sequence halves (1024 steps each),
uniform SPMD, no cross-core communication.

Carry (chunk-1 cores): h_1023 = sum_s A^(1023-s) Bu_prev[s] + A^1024 h0 is
computed WITHOUT any DVE elementwise pass over [N, T] via the rank-space
"Q-trick":  Q = g_prev @ APrev  (PE matmul, g_prev = X_prev @ B1,
APrev[s, n] = A[n]^(1023-s) from 8 Exp activations with per-partition
iota scales), then carry[n] = sum_r B2[r, n] * Q[r, n] (one small DVE
multiply [64, T] + a ones-stationary matmul partition-reduce), reshaped
to per-n-tile columns through a tiny DRAM round-trip.

DVE runs ONLY the 8 own-chunk scans (the true serial recurrence),
split into quarter-batches so CH1/Y of earlier quarters overlap later
scans; the A multiplier is a stride-0 broadcast AP (no materialized
broadcast tensor).

Dtypes: X / W1 / W2 / B2 / pext / gsb / APrev are bf16; scans keep f32
state and H / C1 matmuls run in f32r (exact f32 bits, 1 cycle/row at
free >= 256).
"""

import contextlib
import ctypes
import os
import sys
import types

import numpy as np
from contextlib import ExitStack

import concourse.bass as bass
import concourse.tile as tile
from concourse import bacc, mybir
from concourse.bass_utils import run_bass_kernel_spmd


def _install_ntff_hook():
    try:
        from antenv.axon_hooks import get_axon_ntff_profile_hook  # noqa: F401
        return
    except ImportError:
        pass
    so_path = "/opt/axon/libaxon_pjrt.so"
    hook = None
    if os.path.exists(so_path):
        lib = ctypes.CDLL(so_path)
        if hasattr(lib, "axon_start_nrt_profile"):
            lib.axon_start_nrt_profile.argtypes = [
                ctypes.POINTER(ctypes.c_int64), ctypes.c_size_t]
            lib.axon_start_nrt_profile.restype = ctypes.c_int64
            lib.axon_stop_nrt_profile.argtypes = [ctypes.c_char_p]
            lib.axon_stop_nrt_profile.restype = ctypes.c_int64

            @contextlib.contextmanager
            def _hook(output_dir, device_ids):
                import jax
                jax.devices()
                if device_ids:
                    ids = (ctypes.c_int64 * len(device_ids))(*device_ids)
                    rc = lib.axon_start_nrt_profile(ids, len(device_ids))
                else:
                    rc = lib.axon_start_nrt_profile(None, 0)
                if rc != 0:
                    raise RuntimeError(f"axon_start_nrt_profile rc={rc}")
                try:
                    yield
                finally:
                    n = lib.axon_stop_nrt_profile(str(output_dir).encode())
                    print(f"ntff profile: {n} file(s) -> {output_dir}",
                          file=sys.stderr)

            hook = _hook
    mod = types.ModuleType("antenv.axon_hooks")
    mod.get_axon_ntff_profile_hook = lambda: hook
    mod.set_axon_ntff_profile_hook = lambda h: None
    sys.modules["antenv.axon_hooks"] = mod


_install_ntff_hook()

DT = mybir.dt.float32
HDT = mybir.dt.float32r
MDT = mybir.dt.bfloat16
MNP = mybir.dt.np(MDT)
F32 = np.float32

B, T, D = 4, 2048, 1024
NST, R, KX, OUT = 1024, 64, 5, 1024
TC = 1024          # per-core chunk length
TBL = 512          # pext/ysb half block
TQ = 256           # scan / CH1 / Y quarter block
NT = 2
NQ = 4
PW = 4 + TC + 4

_CACHED_NC = None
LAST_RESULT = None

MULT = mybir.AluOpType.mult
ADD = mybir.AluOpType.add
EXP = mybir.ActivationFunctionType.Exp
LN = mybir.ActivationFunctionType.Ln


def _emit(ctx, tc, io):
    nc = tc.nc
    xo, xp, w1, w2, b2, c1, lnb, yt = io

    wp = ctx.enter_context(tc.tile_pool(name="wp", bufs=1))
    xpool = ctx.enter_context(tc.tile_pool(name="xpool", bufs=1))
    hp = ctx.enter_context(tc.tile_pool(name="hp", bufs=1))
    pp = ctx.enter_context(tc.tile_pool(name="pp", bufs=1))
    yp = ctx.enter_context(tc.tile_pool(name="yp", bufs=8))
    # PSUM (8 banks): psA(2): jp pair -> Q pair -> CH1 quarters
    #                 psB(2): jo/j1/j2 -> Y quarters
    #                 psBu(3): Bu_own quarters   psT(1): transposes/tails/colsum
    psA = ctx.enter_context(tc.tile_pool(name="psA", bufs=2, space="PSUM"))
    psB = ctx.enter_context(tc.tile_pool(name="psB", bufs=2, space="PSUM"))
    psBu = ctx.enter_context(tc.tile_pool(name="psBu", bufs=3, space="PSUM"))
    psT = ctx.enter_context(tc.tile_pool(name="psT", bufs=1, space="PSUM"))

    # ---- DMA (3 queues; X packed as [128, 8k*1024] with 2 transfers) ------
    # sync: s, ident, xpA, xpB, cscr round-trip, yt stores
    # scalar HWDGE: lnbc, w1, xoA, xoB (transfers overlap the Exp chain)
    # gpsimd SWDGE: b2, c1, w2
    c1sb = wp.tile([128, 8 * 64 + 40], HDT, tag="c1", name="c1sb")
    nc.sync.dma_start(c1sb[:], c1[:])
    xpall = xpool.tile([128, 8 * TC], MDT, tag="xpall", name="xpall")
    nc.sync.dma_start(xpall[:, 0:4 * TC], xp[:, 0:4 * TC])
    nc.sync.dma_start(xpall[:, 4 * TC:8 * TC], xp[:, 4 * TC:8 * TC])
    lnbc = wp.tile([128, TC], MDT, tag="lnbc", name="lnbc")
    nc.scalar.dma_start(lnbc[:], lnb[:])
    w1sb = wp.tile([128, 8 * 384 + 64], MDT, tag="w1", name="w1sb")
    nc.scalar.dma_start(w1sb[:], w1[:])
    xoall = xpool.tile([128, 8 * TC], MDT, tag="xoall", name="xoall")
    nc.scalar.dma_start(xoall[:, 0:4 * TC], xo[:, 0:4 * TC])
    nc.sync.dma_start(xoall[:, 4 * TC:8 * TC], xo[:, 4 * TC:8 * TC])
    b2sb = wp.tile([64, NST], MDT, tag="b2", name="b2sb")
    nc.gpsimd.dma_start(b2sb[:], b2[:])
    w2sb = wp.tile([128, 3 * 1024], MDT, tag="w2", name="w2sb")
    nc.gpsimd.dma_start(w2sb[:], w2[:])
    xpsb = [xpall[:, k * TC:(k + 1) * TC] for k in range(8)]
    xosb = [xoall[:, k * TC:(k + 1) * TC] for k in range(8)]

    def w1s(k, j):
        return w1sb[:, k * 384 + j * 128: k * 384 + (j + 1) * 128]

    smf = wp.tile([128, 32], DT, tag="smf", name="smf")
    nc.vector.tensor_copy(smf[:], c1sb[:, 512:544])
    av = smf[:, 0:8]
    ivS = smf[:, 8:16]
    ioS = smf[:, 16:24]
    iotf = smf[:, 24:32]   # 1023 - 128*s - p
    idsb = w1sb[0:64, 3072:3136]

    # ---- tiny prep --------------------------------------------------------
    lnA = wp.tile([128, 8], DT, tag="lnA", name="lnA")
    nc.scalar.activation(lnA[:], av, LN)
    ap1024 = wp.tile([128, 8], DT, tag="ap1024", name="ap1024")
    nc.scalar.activation(ap1024[:], lnA[:], EXP, scale=1024.0)
    ivterm = wp.tile([128, 8], DT, tag="ivterm", name="ivterm")
    nc.vector.scalar_tensor_tensor(ivterm[:], ap1024[:], 1.0, ivS, MULT, MULT)
    nc.vector.scalar_tensor_tensor(ivterm[:], ivterm[:], 1.0, ioS, MULT, ADD)

    # ---- scalar: APrev_s[p, n] = A[n]^(1023 - 128 s - p)  (bf16) ----------
    # pext0 copies are wedged in after exp4 (jo is ready by then) so the
    # Bu_own matmuls aren't gated on the end of the exp chain.
    aprev = [wp.tile([128, TC], MDT, tag=f"apv{s}", name=f"apv{s}")
             for s in range(8)]
    pext = [pp.tile([128, PW], MDT, tag=f"pext{j}", name=f"pext{j}")
            for j in range(3)]

    # ---- PE phase 1: j0 (all jp first so gsb/carry start early) ----------
    jp = [psA.tile([128, TBL], DT, tag="pa", name=f"jp{t}") for t in range(NT)]
    jo = [psB.tile([128, TBL], DT, tag="pb", name=f"jo{t}") for t in range(NT)]
    for k in range(8):
        fl = dict(start=(k == 0), stop=(k == 7), skip_group_check=True)
        for t in range(NT):
            nc.tensor.matmul(jp[t][:], w1s(k, 0),
                             xpsb[k][:, t * TBL:(t + 1) * TBL], **fl)
    for k in range(8):
        fl = dict(start=(k == 0), stop=(k == 7), skip_group_check=True)
        for t in range(NT):
            nc.tensor.matmul(jo[t][:], w1s(k, 0),
                             xosb[k][:, t * TBL:(t + 1) * TBL], **fl)

    # scalar: APrev_s[p, n] = A[n]^(1023 - 128 s - p)  (bf16)
    for s in range(8):
        nc.scalar.activation(aprev[s][:], lnbc[:], EXP,
                             scale=iotf[:, s:s + 1])

    # DVE copies: gsb (j0-prev rows 0:64), pext0 (j0-own), gsbT
    gsb = wp.tile([64, TC], MDT, tag="gsb", name="gsb")
    for t in range(NT):
        nc.vector.tensor_copy(gsb[:, t * TBL:(t + 1) * TBL], jp[t][0:64, :])
    for t in range(NT):
        nc.vector.tensor_copy(pext[0][:, 4 + t * TBL:4 + (t + 1) * TBL],
                              jo[t][:])

    gsbT = []
    for s in range(8):
        pt = psT.tile([128, 64], MDT, tag="tl", name=f"tp{s}")
        nc.tensor.matmul(pt[:], gsb[:, s * 128:(s + 1) * 128], idsb[:],
                         is_transpose=True)
        st = wp.tile([128, 64], MDT, tag=f"gT{s}", name=f"gT{s}")
        nc.vector.tensor_copy(st[:], pt[:])
        gsbT.append(st)

    # Q[r, n] = sum_s gsbT_s^T @ APrev_s  -> psA pair [64, 512]
    qps = [psA.tile([64, TBL], DT, tag="pa", name=f"q{h}") for h in range(2)]
    for h in range(2):
        for s in range(8):
            nc.tensor.matmul(qps[h][:], gsbT[s][:],
                             aprev[s][:, h * TBL:(h + 1) * TBL],
                             start=(s == 0), stop=(s == 7))
    qb = wp.tile([64, TC], MDT, tag="qb", name="qb")
    for h in range(2):
        nc.vector.scalar_tensor_tensor(
            qb[:, h * TBL:(h + 1) * TBL], qps[h][:], 1.0,
            b2sb[:, h * TBL:(h + 1) * TBL], MULT, MULT)
    ones64 = wp.tile([64, 1], MDT, tag="o64", name="ones64")
    nc.vector.memset(ones64[:], 1.0)

    # colsum -> csf -> 8 tiny PE transposes -> iownF [128, 8]
    csf = wp.tile([1, TC], MDT, tag="csf", name="csf")
    for h in range(2):
        cp = psT.tile([1, TBL], DT, tag="tl", name=f"cs{h}")
        nc.tensor.matmul(cp[:], ones64[:], qb[:, h * TBL:(h + 1) * TBL],
                         start=True, stop=True)
        nc.scalar.copy(csf[:, h * TBL:(h + 1) * TBL], cp[:])
    iownT = wp.tile([128, 8], DT, tag="iownT", name="iownT")
    for n in range(8):
        rp = psT.tile([128, 1], MDT, tag="tl", name=f"rt{n}")
        nc.tensor.matmul(rp[:], csf[0:1, n * 128:(n + 1) * 128],
                         idsb[0:1, 0:1], is_transpose=True)
        nc.vector.tensor_copy(iownT[:, n:n + 1], rp[:])
    iownF = wp.tile([128, 8], DT, tag="iownF", name="iownF")
    nc.vector.scalar_tensor_tensor(iownF[:], iownT[:], 1.0, ivterm[:],
                                   MULT, ADD)

    # ---- Bu_own + scans start as soon as jo/pext0 land --------------------
    hsb = [hp.tile([128, TC], HDT, tag=f"h{n}", name=f"h{n}") for n in range(8)]
    buo = {}

    def emit_buo(n, q):
        p = psBu.tile([128, TQ], DT, tag="bu", name=f"buo{n}_{q}")
        nc.tensor.matmul(p[:], b2sb[:, n * 128:(n + 1) * 128],
                         pext[0][0:64, 4 + q * TQ:4 + (q + 1) * TQ],
                         start=True, stop=True)
        buo[(n, q)] = p

    def emit_scan(n, q):
        init = iownF[:, n:n + 1] if q == 0 else hsb[n][:, q * TQ - 1:q * TQ]
        nc.vector.tensor_tensor_scan(
            hsb[n][:, q * TQ:(q + 1) * TQ],
            av[:, n:n + 1].broadcast_to([128, TQ]),
            buo[(n, q)][:], init, MULT, ADD)

    for n in range(8):
        emit_buo(n, 0)

    # ---- j1/j2/tails (needed only by Y, from ~q0 CH1 time) ----------------
    j1 = [psB.tile([128, TBL], DT, tag="pb", name=f"j1_{t}") for t in range(NT)]
    j2 = [psB.tile([128, TBL], DT, tag="pb", name=f"j2_{t}") for t in range(NT)]
    tails = psT.tile([128, 8], DT, tag="tl", name="tails")

    def emit_j(jt, j, t):
        for k in range(8):
            nc.tensor.matmul(jt[t][:], w1s(k, j),
                             xosb[k][:, t * TBL:(t + 1) * TBL],
                             start=(k == 0), stop=(k == 7))

    def emit_tail(j):
        for k in range(8):
            nc.tensor.matmul(tails[:, (j - 1) * 4:j * 4], w1s(k, j),
                             xpsb[k][:, TC - 4:TC],
                             start=(k == 0), stop=(k == 7))

    emit_j(j1, 1, 0)
    emit_j(j1, 1, 1)
    emit_tail(1)
    emit_j(j2, 2, 0)
    emit_j(j2, 2, 1)
    emit_tail(2)

    for n in range(8):
        emit_scan(n, 0)
        emit_buo(n, 1)

    # scalar: pext copies for j1/j2/tails
    for t in range(NT):
        nc.scalar.copy(pext[1][0:64, 5 + t * TBL:5 + (t + 1) * TBL],
                       j1[t][0:64, :])
        nc.scalar.copy(pext[1][64:128, 6 + t * TBL:6 + (t + 1) * TBL],
                       j1[t][64:128, :])
    for t in range(NT):
        nc.scalar.copy(pext[2][0:64, 7 + t * TBL:7 + (t + 1) * TBL],
                       j2[t][0:64, :])
        nc.scalar.copy(pext[2][64:128, 8 + t * TBL:8 + (t + 1) * TBL],
                       j2[t][64:128, :])
    nc.scalar.copy(pext[1][0:64, 4:5], tails[0:64, 3:4])
    nc.scalar.copy(pext[1][64:128, 4:6], tails[64:128, 2:4])
    nc.scalar.copy(pext[2][0:64, 4:7], tails[0:64, 5:8])
    nc.scalar.copy(pext[2][64:128, 4:8], tails[64:128, 4:8])

    # ---- per-quarter: remaining scans, CH1, Y -----------------------------
    ysb = {}
    for q in range(NQ):
        if q >= 1:  # scans for quarter q (q0 emitted above)
            for n in range(8):
                emit_scan(n, q)
                if q < NQ - 1:
                    emit_buo(n, q + 1)
        c_ps = psA.tile([64, TQ], DT, tag="pa", name=f"c_ps{q}")
        for n in range(8):
            nc.tensor.matmul(c_ps[:], c1sb[:, n * 64:(n + 1) * 64],
                             hsb[n][:, q * TQ:(q + 1) * TQ],
                             start=(n == 0), stop=(n == 7))
        nc.scalar.copy(pext[0][0:64, 4 + q * TQ:4 + (q + 1) * TQ], c_ps[:])

        for o in range(8):
            y_ps = psB.tile([128, TQ], DT, tag="pb", name=f"y_ps{o}_{q}")
            for m in range(3):
                nc.tensor.matmul(y_ps[:],
                                 w2sb[:, m * 1024 + o * 128:
                                      m * 1024 + (o + 1) * 128],
                                 pext[m][:, 4 + q * TQ:4 + (q + 1) * TQ],
                                 start=(m == 0), stop=(m == 2))
            half, qh = divmod(q, 2)
            if qh == 0:
                ysb[(o, half)] = yp.tile([128, TBL], MDT, tag="y",
                                         name=f"y{o}_{half}")
            dst = ysb[(o, half)][:, qh * TQ:(qh + 1) * TQ]
            if q == 3 and o % 2 == 1:
                nc.vector.tensor_copy(dst, y_ps[:])
            else:
                nc.scalar.copy(dst, y_ps[:])
            if qh == 1:
                nc.sync.dma_start(
                    yt[o * 128:(o + 1) * 128, half * TBL:(half + 1) * TBL],
                    ysb[(o, half)][:])


def _build():
    nc = bacc.Bacc("TRN2", target_bir_lowering=False, debug=False,
                   num_devices=8)
    xo = nc.dram_tensor("xo", [128, 8 * TC], MDT, kind="ExternalInput").ap()
    xp = nc.dram_tensor("xp", [128, 8 * TC], MDT, kind="ExternalInput").ap()
    w1 = nc.dram_tensor("w1", [128, 8 * 384 + 64], MDT, kind="ExternalInput").ap()
    w2 = nc.dram_tensor("w2", [128, 3 * 1024], MDT, kind="ExternalInput").ap()
    b2 = nc.dram_tensor("b2", [R, NST], MDT, kind="ExternalInput").ap()
    c1 = nc.dram_tensor("c1", [128, 8 * 64 + 40], HDT, kind="ExternalInput").ap()
    lnb = nc.dram_tensor("lnb", [128, TC], MDT, kind="ExternalInput").ap()
    yt = nc.dram_tensor("yt", [OUT, TC], MDT, kind="ExternalOutput").ap()

    with tile.TileContext(nc) as tc, ExitStack() as ctx:
        _emit(ctx, tc, (xo, xp, w1, w2, b2, c1, lnb, yt))
    nc.compile()
    return nc


def _get_nc():
    global _CACHED_NC
    if _CACHED_NC is None:
        _CACHED_NC = _build()
    return _CACHED_NC


def _tile_pack(a, p=128):
    k = a.shape[0] // p
    return np.ascontiguousarray(
        np.transpose(a.reshape(k, p, -1), (1, 0, 2)).reshape(p, -1))


def kernel(inputs, h0, A, B1, B2, C1, C2, M1, M2):
    global LAST_RESULT
    X = np.asarray(inputs, dtype=F32)
    h0 = np.asarray(h0, dtype=F32)
    A = np.asarray(A, dtype=F32)
    W1 = np.concatenate(
        [np.asarray(B1, dtype=F32)]
        + [np.ascontiguousarray(np.asarray(M1, dtype=F32)[:, :, k].T)
           for k in range(KX)], axis=1)
    W2 = np.concatenate(
        [np.asarray(C2, dtype=F32)]
        + [np.ascontiguousarray(np.asarray(M2, dtype=F32)[:, :, k].T)
           for k in range(KX)], axis=0)
    w1c = np.concatenate(
        [_tile_pack(W1).astype(MNP),
         np.vstack([np.eye(64, dtype=F32),
                    np.zeros((64, 64), F32)]).astype(MNP)], axis=1)
    w2c = _tile_pack(W2).astype(MNP)
    b2c = np.ascontiguousarray(np.asarray(B2, dtype=F32)).astype(MNP)
    c1c = _tile_pack(np.asarray(C1, dtype=F32))
    avT = _tile_pack(A.reshape(-1, 1))
    h0T = _tile_pack(h0.reshape(-1, 1))
    zT = np.zeros((128, 8), F32)
    s8 = np.arange(8)
    p128 = np.arange(128)
    iotf = (1023.0 - 128.0 * s8[None, :] - p128[:, None]).astype(F32)
    lnbc = np.ascontiguousarray(
        np.broadcast_to(np.log(A)[None, :], (128, NST))).astype(MNP)

    xz = np.zeros((128, 8 * TC), MNP)
    in_maps = []
    for c in range(8):
        b, half = divmod(c, 2)
        xoc = _tile_pack(
            np.ascontiguousarray(X[b, half * TC:(half + 1) * TC, :].T)
        ).astype(MNP)
        if half == 0:
            xpc = xz
            ivT, ioT = zT, h0T
        else:
            xpc = _tile_pack(
                np.ascontiguousarray(X[b, 0:TC, :].T)).astype(MNP)
            ivT, ioT = h0T, zT
        ones8 = np.ones((128, 8), F32)
        c1x = np.ascontiguousarray(
            np.concatenate([c1c, avT, ivT, ioT, iotf, ones8], axis=1))
        in_maps.append({"xo": xoc, "xp": xpc, "w1": w1c, "w2": w2c,
                        "b2": b2c, "c1": c1x, "lnb": lnbc})

    nc = _get_nc()
    trace = bool(int(os.environ.get("KERNEL_TRACE", "0")))
    LAST_RESULT = run_bass_kernel_spmd(nc, in_maps, core_ids=list(range(8)),
                                       trace=trace)
    Y = np.empty((B, T, OUT), F32)
    for c in range(8):
        b, half = divmod(c, 2)
        Y[b, half * TC:(half + 1) * TC, :] = \
            LAST_RESULT.results[c]["yt"].T.astype(F32)
    return Y


# revision 22
# speedup vs baseline: 1.1736x; 1.1142x over previous
"""Trainium2 Bass kernel for nn_LDS_LR: low-rank LDS + AR low-rank correction.

Math (per batch b):
    Bu   = X @ B1 @ B2                      # [T, N] rank-64 input projection
    h_t  = A * h_{t-1} + Bu_t               # diagonal recurrence, h_{-1} = h0
    lds  = H @ C1 @ C2                      # [T, O] rank-64 output projection
    proj = einsum('ti,rik->trk', X, M1)     # [T, R, KX]
    ar_t = sum_k M2[:,:,k] @ proj[t-k,:,k]  # AR with KX=5 taps
    Y    = lds + ar

Sharding: 8 cores = 4 batches x 2 sequence halves (1024 steps each),
uniform SPMD, no cross-core communication.

Carry (chunk-1 cores): h_1023 = sum_s A^(1023-s) Bu_prev[s] + A^1024 h0 is
computed WITHOUT any DVE elementwise pass over [N, T] via the rank-space
"Q-trick":  Q = g_prev @ APrev  (PE matmul, g_prev = X_prev @ B1,
APrev[s, n] = A[n]^(1023-s) from 8 Exp activations with per-partition
iota scales), then carry[n] = sum_r B2[r, n] * Q[r, n] (one small DVE
multiply [64, T] + a ones-stationary matmul partition-reduce), reshaped
to per-n-tile columns through a tiny DRAM round-trip.

DVE runs ONLY the 8 own-chunk scans (the true serial recurrence),
split into quarter-batches so CH1/Y of earlier quarters overlap later
scans; the A multiplier is a stride-0 broadcast AP (no materialized
broadcast tensor).

Dtypes: X / W1 / W2 / B2 / pext / gsb / APrev are bf16; scans keep f32
state and H / C1 matmuls run in f32r (exact f32 bits, 1 cycle/row at
free >= 256).
"""

import contextlib
import ctypes
import os
import sys
import types

import numpy as np
from contextlib import ExitStack

import concourse.bass as bass
import concourse.tile as tile
from concourse import bacc, mybir
from concourse.bass_utils import run_bass_kernel_spmd


def _install_ntff_hook():
    try:
        from antenv.axon_hooks import get_axon_ntff_profile_hook  # noqa: F401
        return
    except ImportError:
        pass
    so_path = "/opt/axon/libaxon_pjrt.so"
    hook = None
    if os.path.exists(so_path):
        lib = ctypes.CDLL(so_path)
        if hasattr(lib, "axon_start_nrt_profile"):
            lib.axon_start_nrt_profile.argtypes = [
                ctypes.POINTER(ctypes.c_int64), ctypes.c_size_t]
            lib.axon_start_nrt_profile.restype = ctypes.c_int64
            lib.axon_stop_nrt_profile.argtypes = [ctypes.c_char_p]
            lib.axon_stop_nrt_profile.restype = ctypes.c_int64

            @contextlib.contextmanager
            def _hook(output_dir, device_ids):
                import jax
                jax.devices()
                if device_ids:
                    ids = (ctypes.c_int64 * len(device_ids))(*device_ids)
                    rc = lib.axon_start_nrt_profile(ids, len(device_ids))
                else:
                    rc = lib.axon_start_nrt_profile(None, 0)
                if rc != 0:
                    raise RuntimeError(f"axon_start_nrt_profile rc={rc}")
                try:
                    yield
                finally:
                    n = lib.axon_stop_nrt_profile(str(output_dir).encode())
                    print(f"ntff profile: {n} file(s) -> {output_dir}",
                          file=sys.stderr)

            hook = _hook
    mod = types.ModuleType("antenv.axon_hooks")
    mod.get_axon_ntff_profile_hook = lambda: hook
    mod.set_axon_ntff_profile_hook = lambda h: None
    sys.modules["antenv.axon_hooks"] = mod


_install_ntff_hook()

DT = mybir.dt.float32
HDT = mybir.dt.float32r
MDT = mybir.dt.bfloat16
MNP = mybir.dt.np(MDT)
F32 = np.float32

B, T, D = 4, 2048, 1024
NST, R, KX, OUT = 1024, 64, 5, 1024
TC = 1024          # per-core chunk length
TBL = 512          # pext/ysb half block
TQ = 256           # scan / CH1 / Y quarter block
NT = 2
NQ = 4
PW = 4 + TC + 4

_CACHED_NC = None
LAST_RESULT = None

MULT = mybir.AluOpType.mult
ADD = mybir.AluOpType.add
EXP = mybir.ActivationFunctionType.Exp
LN = mybir.ActivationFunctionType.Ln


def _emit(ctx, tc, io):
    nc = tc.nc
    xo, xp, w1, w2, b2, c1, lnb, yt = io

    wp = ctx.enter_context(tc.tile_pool(name="wp", bufs=1))
    xpool = ctx.enter_context(tc.tile_pool(name="xpool", bufs=1))
    hp = ctx.enter_context(tc.tile_pool(name="hp", bufs=1))
    pp = ctx.enter_context(tc.tile_pool(name="pp", bufs=1))
    yp = ctx.enter_context(tc.tile_pool(name="yp", bufs=8))
    # PSUM (8 banks): psA(2): jp pair -> Q pair -> CH1 quarters
    #                 psB(2): jo/j1/j2 -> Y quarters
    #                 psBu(3): Bu_own quarters   psT(1): transposes/tails/colsum
    psA = ctx.enter_context(tc.tile_pool(name="psA", bufs=2, space="PSUM"))
    psB = ctx.enter_context(tc.tile_pool(name="psB", bufs=2, space="PSUM"))
    psBu = ctx.enter_context(tc.tile_pool(name="psBu", bufs=3, space="PSUM"))
    psT = ctx.enter_context(tc.tile_pool(name="psT", bufs=1, space="PSUM"))

    # ---- DMA (3 queues; X packed as [128, 8k*1024] with 2 transfers) ------
    # sync: s, ident, xpA, xpB, cscr round-trip, yt stores
    # scalar HWDGE: lnbc, w1, xoA, xoB (transfers overlap the Exp chain)
    # gpsimd SWDGE: b2, c1, w2
    c1sb = wp.tile([128, 8 * 64 + 40], HDT, tag="c1", name="c1sb")
    nc.sync.dma_start(c1sb[:], c1[:])
    xpall = xpool.tile([128, 8 * TC], MDT, tag="xpall", name="xpall")
    nc.sync.dma_start(xpall[:, 0:4 * TC], xp[:, 0:4 * TC])
    nc.sync.dma_start(xpall[:, 4 * TC:8 * TC], xp[:, 4 * TC:8 * TC])
    lnbc = wp.tile([128, TC], MDT, tag="lnbc", name="lnbc")
    nc.scalar.dma_start(lnbc[:], lnb[:])
    w1sb = wp.tile([128, 8 * 384 + 64], MDT, tag="w1", name="w1sb")
    nc.scalar.dma_start(w1sb[:], w1[:])
    xoall = xpool.tile([128, 8 * TC], MDT, tag="xoall", name="xoall")
    nc.scalar.dma_start(xoall[:, 0:4 * TC], xo[:, 0:4 * TC])
    nc.sync.dma_start(xoall[:, 4 * TC:6 * TC], xo[:, 4 * TC:6 * TC])
    nc.scalar.dma_start(xoall[:, 6 * TC:7 * TC], xo[:, 6 * TC:7 * TC])
    b2sb = wp.tile([64, NST], MDT, tag="b2", name="b2sb")
    nc.gpsimd.dma_start(b2sb[:], b2[:])
    nc.gpsimd.dma_start(xoall[:, 7 * TC:8 * TC], xo[:, 7 * TC:8 * TC])
    w2sb = wp.tile([128, 3 * 1024], MDT, tag="w2", name="w2sb")
    nc.gpsimd.dma_start(w2sb[:], w2[:])
    xpsb = [xpall[:, k * TC:(k + 1) * TC] for k in range(8)]
    xosb = [xoall[:, k * TC:(k + 1) * TC] for k in range(8)]

    def w1s(k, j):
        return w1sb[:, k * 384 + j * 128: k * 384 + (j + 1) * 128]

    smf = wp.tile([128, 32], DT, tag="smf", name="smf")
    nc.vector.tensor_copy(smf[:], c1sb[:, 512:544])
    av = smf[:, 0:8]
    ivS = smf[:, 8:16]
    ioS = smf[:, 16:24]
    iotf = smf[:, 24:32]   # 1023 - 128*s - p
    idsb = w1sb[0:64, 3072:3136]

    # ---- tiny prep --------------------------------------------------------
    lnA = wp.tile([128, 8], DT, tag="lnA", name="lnA")
    nc.scalar.activation(lnA[:], av, LN)
    ap1024 = wp.tile([128, 8], DT, tag="ap1024", name="ap1024")
    nc.scalar.activation(ap1024[:], lnA[:], EXP, scale=1024.0)
    ivterm = wp.tile([128, 8], DT, tag="ivterm", name="ivterm")
    nc.vector.scalar_tensor_tensor(ivterm[:], ap1024[:], 1.0, ivS, MULT, MULT)
    nc.vector.scalar_tensor_tensor(ivterm[:], ivterm[:], 1.0, ioS, MULT, ADD)

    # ---- scalar: APrev_s[p, n] = A[n]^(1023 - 128 s - p)  (bf16) ----------
    # pext0 copies are wedged in after exp4 (jo is ready by then) so the
    # Bu_own matmuls aren't gated on the end of the exp chain.
    aprev = [wp.tile([128, TC], MDT, tag=f"apv{s}", name=f"apv{s}")
             for s in range(8)]
    pext = [pp.tile([128, PW], MDT, tag=f"pext{j}", name=f"pext{j}")
            for j in range(3)]

    # ---- PE phase 1: j0 (all jp first so gsb/carry start early) ----------
    jp = [psA.tile([128, TBL], DT, tag="pa", name=f"jp{t}") for t in range(NT)]
    jo = [psB.tile([128, TBL], DT, tag="pb", name=f"jo{t}") for t in range(NT)]
    for k in range(8):
        fl = dict(start=(k == 0), stop=(k == 7), skip_group_check=True)
        for t in range(NT):
            nc.tensor.matmul(jp[t][:], w1s(k, 0),
                             xpsb[k][:, t * TBL:(t + 1) * TBL], **fl)
    for k in range(8):
        fl = dict(start=(k == 0), stop=(k == 7), skip_group_check=True)
        for t in range(NT):
            nc.tensor.matmul(jo[t][:], w1s(k, 0),
                             xosb[k][:, t * TBL:(t + 1) * TBL], **fl)

    # scalar: APrev_s[p, n] = A[n]^(1023 - 128 s - p)  (bf16)
    for s in range(8):
        nc.scalar.activation(aprev[s][:], lnbc[:], EXP,
                             scale=iotf[:, s:s + 1])

    # DVE copies: gsb (j0-prev rows 0:64), pext0 (j0-own), gsbT
    gsb = wp.tile([64, TC], MDT, tag="gsb", name="gsb")
    for t in range(NT):
        nc.vector.tensor_copy(gsb[:, t * TBL:(t + 1) * TBL], jp[t][0:64, :])
    for t in range(NT):
        nc.vector.tensor_copy(pext[0][:, 4 + t * TBL:4 + (t + 1) * TBL],
                              jo[t][:])

    gsbT = []
    for s in range(8):
        pt = psT.tile([128, 64], MDT, tag="tl", name=f"tp{s}")
        nc.tensor.matmul(pt[:], gsb[:, s * 128:(s + 1) * 128], idsb[:],
                         is_transpose=True)
        st = wp.tile([128, 64], MDT, tag=f"gT{s}", name=f"gT{s}")
        nc.vector.tensor_copy(st[:], pt[:])
        gsbT.append(st)

    # Q[r, n] = sum_s gsbT_s^T @ APrev_s  -> psA pair [64, 512]
    qps = [psA.tile([64, TBL], DT, tag="pa", name=f"q{h}") for h in range(2)]
    for h in range(2):
        for s in range(8):
            nc.tensor.matmul(qps[h][:], gsbT[s][:],
                             aprev[s][:, h * TBL:(h + 1) * TBL],
                             start=(s == 0), stop=(s == 7))
    qb = wp.tile([64, TC], MDT, tag="qb", name="qb")
    for h in range(2):
        nc.vector.scalar_tensor_tensor(
            qb[:, h * TBL:(h + 1) * TBL], qps[h][:], 1.0,
            b2sb[:, h * TBL:(h + 1) * TBL], MULT, MULT)
    ones64 = wp.tile([64, 1], MDT, tag="o64", name="ones64")
    nc.vector.memset(ones64[:], 1.0)

    # colsum -> csf -> 8 tiny PE transposes -> iownF [128, 8]
    csf = wp.tile([1, TC], MDT, tag="csf", name="csf")
    for h in range(2):
        cp = psT.tile([1, TBL], DT, tag="tl", name=f"cs{h}")
        nc.tensor.matmul(cp[:], ones64[:], qb[:, h * TBL:(h + 1) * TBL],
                         start=True, stop=True)
        nc.scalar.copy(csf[:, h * TBL:(h + 1) * TBL], cp[:])
    iownT = wp.tile([128, 8], DT, tag="iownT", name="iownT")
    for n in range(8):
        rp = psT.tile([128, 1], MDT, tag="tl", name=f"rt{n}")
        nc.tensor.matmul(rp[:], csf[0:1, n * 128:(n + 1) * 128],
                         idsb[0:1, 0:1], is_transpose=True)
        nc.vector.tensor_copy(iownT[:, n:n + 1], rp[:])
    iownF = wp.tile([128, 8], DT, tag="iownF", name="iownF")
    nc.vector.scalar_tensor_tensor(iownF[:], iownT[:], 1.0, ivterm[:],
                                   MULT, ADD)

    # ---- Bu_own + scans start as soon as jo/pext0 land --------------------
    hsb = [hp.tile([128, TC], HDT, tag=f"h{n}", name=f"h{n}") for n in range(8)]
    buo = {}

    def emit_buo(n, q):
        p = psBu.tile([128, TQ], DT, tag="bu", name=f"buo{n}_{q}")
        nc.tensor.matmul(p[:], b2sb[:, n * 128:(n + 1) * 128],
                         pext[0][0:64, 4 + q * TQ:4 + (q + 1) * TQ],
                         start=True, stop=True)
        buo[(n, q)] = p

    def emit_scan(n, q):
        init = iownF[:, n:n + 1] if q == 0 else hsb[n][:, q * TQ - 1:q * TQ]
        nc.vector.tensor_tensor_scan(
            hsb[n][:, q * TQ:(q + 1) * TQ],
            av[:, n:n + 1].broadcast_to([128, TQ]),
            buo[(n, q)][:], init, MULT, ADD)

    for n in range(8):
        emit_buo(n, 0)

    # ---- j1/j2/tails (needed only by Y, from ~q0 CH1 time) ----------------
    j1 = [psB.tile([128, TBL], DT, tag="pb", name=f"j1_{t}") for t in range(NT)]
    j2 = [psB.tile([128, TBL], DT, tag="pb", name=f"j2_{t}") for t in range(NT)]
    tails = psT.tile([128, 8], DT, tag="tl", name="tails")

    def emit_j(jt, j, t):
        for k in range(8):
            nc.tensor.matmul(jt[t][:], w1s(k, j),
                             xosb[k][:, t * TBL:(t + 1) * TBL],
                             start=(k == 0), stop=(k == 7))

    def emit_tail(j):
        for k in range(8):
            nc.tensor.matmul(tails[:, (j - 1) * 4:j * 4], w1s(k, j),
                             xpsb[k][:, TC - 4:TC],
                             start=(k == 0), stop=(k == 7))

    emit_j(j1, 1, 0)
    emit_j(j1, 1, 1)
    emit_tail(1)
    emit_j(j2, 2, 0)
    emit_j(j2, 2, 1)
    emit_tail(2)

    for n in range(8):
        emit_scan(n, 0)
        emit_buo(n, 1)

    # scalar: pext copies for j1/j2/tails
    for t in range(NT):
        nc.scalar.copy(pext[1][0:64, 5 + t * TBL:5 + (t + 1) * TBL],
                       j1[t][0:64, :])
        nc.scalar.copy(pext[1][64:128, 6 + t * TBL:6 + (t + 1) * TBL],
                       j1[t][64:128, :])
    for t in range(NT):
        nc.scalar.copy(pext[2][0:64, 7 + t * TBL:7 + (t + 1) * TBL],
                       j2[t][0:64, :])
        nc.scalar.copy(pext[2][64:128, 8 + t * TBL:8 + (t + 1) * TBL],
                       j2[t][64:128, :])
    nc.scalar.copy(pext[1][0:64, 4:5], tails[0:64, 3:4])
    nc.scalar.copy(pext[1][64:128, 4:6], tails[64:128, 2:4])
    nc.scalar.copy(pext[2][0:64, 4:7], tails[0:64, 5:8])
    nc.scalar.copy(pext[2][64:128, 4:8], tails[64:128, 4:8])

    # ---- per-quarter: remaining scans, CH1, Y -----------------------------
    ysb = {}
    for q in range(NQ):
        if q >= 1:  # scans for quarter q (q0 emitted above)
            for n in range(8):
                emit_scan(n, q)
                if q < NQ - 1:
                    emit_buo(n, q + 1)
        c_ps = psA.tile([64, TQ], DT, tag="pa", name=f"c_ps{q}")
        for n in range(8):
            nc.tensor.matmul(c_ps[:], c1sb[:, n * 64:(n + 1) * 64],
                             hsb[n][:, q * TQ:(q + 1) * TQ],
                             start=(n == 0), stop=(n == 7))
        nc.scalar.copy(pext[0][0:64, 4 + q * TQ:4 + (q + 1) * TQ], c_ps[:])

        for o in range(8):
            y_ps = psB.tile([128, TQ], DT, tag="pb", name=f"y_ps{o}_{q}")
            for m in range(3):
                nc.tensor.matmul(y_ps[:],
                                 w2sb[:, m * 1024 + o * 128:
                                      m * 1024 + (o + 1) * 128],
                                 pext[m][:, 4 + q * TQ:4 + (q + 1) * TQ],
                                 start=(m == 0), stop=(m == 2))
            half, qh = divmod(q, 2)
            if qh == 0:
                ysb[(o, half)] = yp.tile([128, TBL], MDT, tag="y",
                                         name=f"y{o}_{half}")
            dst = ysb[(o, half)][:, qh * TQ:(qh + 1) * TQ]
            if q == 3 and o % 2 == 1:
                nc.vector.tensor_copy(dst, y_ps[:])
            else:
                nc.scalar.copy(dst, y_ps[:])
            if qh == 1:
                nc.sync.dma_start(
                    yt[o * 128:(o + 1) * 128, half * TBL:(half + 1) * TBL],
                    ysb[(o, half)][:])


def _build():
    nc = bacc.Bacc("TRN2", target_bir_lowering=False, debug=False,
                   num_devices=8)
    xo = nc.dram_tensor("xo", [128, 8 * TC], MDT, kind="ExternalInput").ap()
    xp = nc.dram_tensor("xp", [128, 8 * TC], MDT, kind="ExternalInput").ap()
    w1 = nc.dram_tensor("w1", [128, 8 * 384 + 64], MDT, kind="ExternalInput").ap()
    w2 = nc.dram_tensor("w2", [128, 3 * 1024], MDT, kind="ExternalInput").ap()
    b2 = nc.dram_tensor("b2", [R, NST], MDT, kind="ExternalInput").ap()
    c1 = nc.dram_tensor("c1", [128, 8 * 64 + 40], HDT, kind="ExternalInput").ap()
    lnb = nc.dram_tensor("lnb", [128, TC], MDT, kind="ExternalInput").ap()
    yt = nc.dram_tensor("yt", [OUT, TC], MDT, kind="ExternalOutput").ap()

    with tile.TileContext(nc) as tc, ExitStack() as ctx:
        _emit(ctx, tc, (xo, xp, w1, w2, b2, c1, lnb, yt))
    nc.compile()
    return nc


def _get_nc():
    global _CACHED_NC
    if _CACHED_NC is None:
        _CACHED_NC = _build()
    return _CACHED_NC


def _tile_pack(a, p=128):
    k = a.shape[0] // p
    return np.ascontiguousarray(
        np.transpose(a.reshape(k, p, -1), (1, 0, 2)).reshape(p, -1))


def kernel(inputs, h0, A, B1, B2, C1, C2, M1, M2):
    global LAST_RESULT
    X = np.asarray(inputs, dtype=F32)
    h0 = np.asarray(h0, dtype=F32)
    A = np.asarray(A, dtype=F32)
    W1 = np.concatenate(
        [np.asarray(B1, dtype=F32)]
        + [np.ascontiguousarray(np.asarray(M1, dtype=F32)[:, :, k].T)
           for k in range(KX)], axis=1)
    W2 = np.concatenate(
        [np.asarray(C2, dtype=F32)]
        + [np.ascontiguousarray(np.asarray(M2, dtype=F32)[:, :, k].T)
           for k in range(KX)], axis=0)
    w1c = np.concatenate(
        [_tile_pack(W1).astype(MNP),
         np.vstack([np.eye(64, dtype=F32),
                    np.zeros((64, 64), F32)]).astype(MNP)], axis=1)
    w2c = _tile_pack(W2).astype(MNP)
    b2c = np.ascontiguousarray(np.asarray(B2, dtype=F32)).astype(MNP)
    c1c = _tile_pack(np.asarray(C1, dtype=F32))
    avT = _tile_pack(A.reshape(-1, 1))
    h0T = _tile_pack(h0.reshape(-1, 1))
    zT = np.zeros((128, 8), F32)
    s8 = np.arange(8)
    p128 = np.arange(128)
    iotf = (1023.0 - 128.0 * s8[None, :] - p128[:, None]).astype(F32)
    lnbc = np.ascontiguousarray(
        np.broadcast_to(np.log(A)[None, :], (128, NST))).astype(MNP)

    xz = np.zeros((128, 8 * TC), MNP)
    in_maps = []
    for c in range(8):
        b, half = divmod(c, 2)
        xoc = _tile_pack(
            np.ascontiguousarray(X[b, half * TC:(half + 1) * TC, :].T)
        ).astype(MNP)
        if half == 0:
            xpc = xz
            ivT, ioT = zT, h0T
        else:
            xpc = _tile_pack(
                np.ascontiguousarray(X[b, 0:TC, :].T)).astype(MNP)
            ivT, ioT = h0T, zT
        ones8 = np.ones((128, 8), F32)
        c1x = np.ascontiguousarray(
            np.concatenate([c1c, avT, ivT, ioT, iotf, ones8], axis=1))
        in_maps.append({"xo": xoc, "xp": xpc, "w1": w1c, "w2": w2c,
                        "b2": b2c, "c1": c1x, "lnb": lnbc})

    nc = _get_nc()
    trace = bool(int(os.environ.get("KERNEL_TRACE", "0")))
    LAST_RESULT = run_bass_kernel_spmd(nc, in_maps, core_ids=list(range(8)),
                                       trace=trace)
    Y = np.empty((B, T, OUT), F32)
    for c in range(8):
        b, half = divmod(c, 2)
        Y[b, half * TC:(half + 1) * TC, :] = \
            LAST_RESULT.results[c]["yt"].T.astype(F32)
    return Y
